# revision 19
# baseline (speedup 1.0000x reference)
"""Causal multi-head attention on 8 Trainium2 NeuronCores.

Problem: x[B=4,S=2048,E=1024], Wq/Wk/Wv[H=16,E,D=64], Wo[E,E], bo[E].
  out = softmax_causal(q k^T / sqrt(D)) v, heads concat, @ Wo.T + bo

Sharding (tensor parallel over heads, data parallel over batch):
  core c -> (batch b = c//2, head-group g = c%2 of 8 heads).
  Each core: QKV projections + attention for its 8 heads of its batch;
  pairwise AllGather (cores 2b, 2b+1) of the normalized attention outputs
  (bf16) after each sequence chunk; each core then computes half of the
  output-projection columns (e in [512g, 512g+512)) for its batch.
  Host only slices inputs / concatenates+transposes outputs.

Kernel internals (per core) -- fused chunk pipeline:
  - Everything bf16 into the PE (f32 PSUM accumulation).  Activations kept
    transposed: xT[E,S], QT/KT[dg,S], scoresT[t,s]; softmax denominator
    via a ones-column appended to V (AV matmul row 64).
  - Per sequence chunk c (512 queries): project K/V tiles and Q for chunk
    c+1 *interleaved into* chunk c's attention j-loop (a closure queue
    drains ~2 PE ops per j-step), so the PE stays busy during the
    ScalarE-exp-bound stretches and HAM keeps the PE at full clock.
  - Scores for a key tile are computed for both head-halves into one
    2-bank PSUM group [128,2,512]; a single N=1024 ACTIVATE exponentiates
    both halves (amortizes the 352-cycle ACT pipe fill).
  - Causality: query chunks of 512, key tiles of 128; diagonal tiles
    column-restricted for scores/AV and cleaned with precomputed
    [128,512] shifted-triangular masks after the (full-width) exp.
  - Out-projection of chunk c-1 and the AllGather overlap chunk c's
    attention.
"""

import os
import sys
from collections import deque

for _p in ("/opt/trn_rl_repo", "/root/.axon_site/_ro/trn_rl_repo"):
    if os.path.isdir(_p) and _p not in sys.path:
        sys.path.append(_p)

import numpy as np
import ml_dtypes

import concourse.bass as bass
import concourse.mybir as mybir
import concourse.tile as tile
from concourse import bacc

B, S, E, H, D = 4, 2048, 1024, 16, 64
NCORES = 8
G = 2  # head groups
HL = H // G  # heads per core = 8
DG = HL * D  # local head dim = 512
EH = E // G  # output-projection columns per core = 512
P = 128
SC = 512  # sequence chunk
NSC = S // SC  # 4
NT = S // P  # 16 key tiles
ET = E // P  # 8 embedding tiles
ND = DG // P  # 4 head-pair tiles
SCALE = 1.0 / np.sqrt(D)

F32 = mybir.dt.float32
BF16 = mybir.dt.bfloat16

_CACHE = {}


def _build_nc():
    nc = bacc.Bacc("TRN2", target_bir_lowering=False, debug=False, num_devices=NCORES)

    xT = nc.dram_tensor("xT", [E, S], BF16, kind="ExternalInput")
    wq = nc.dram_tensor("wq", [E, DG], BF16, kind="ExternalInput")
    wk = nc.dram_tensor("wk", [E, DG], BF16, kind="ExternalInput")
    wv = nc.dram_tensor("wv", [E, DG], BF16, kind="ExternalInput")
    woT = nc.dram_tensor("woT", [E, EH], BF16, kind="ExternalInput")
    bo = nc.dram_tensor("bo", [P, EH // P], F32, kind="ExternalInput")
    masks = nc.dram_tensor("masks", [P, 4, SC], BF16, kind="ExternalInput")
    sel = nc.dram_tensor("sel", [2, P], BF16, kind="ExternalInput")
    outT = nc.dram_tensor("outT", [EH, S], F32, kind="ExternalOutput")
    dbg = None
    if os.environ.get("K_DEBUG_ATTN"):
        dbg = nc.dram_tensor("dbg", [NSC, DG, SC], BF16, kind="ExternalOutput")

    with tile.TileContext(nc) as tc:
        with (
            tc.tile_pool(name="persist", bufs=1) as persist,
            tc.tile_pool(name="dram", bufs=1, space="DRAM") as dram,
            tc.tile_pool(name="xp", bufs=2) as xp,
            tc.tile_pool(name="qp", bufs=2) as qp,
            tc.tile_pool(name="exp", bufs=4) as expp,
            tc.tile_pool(name="attn", bufs=2) as attnp,
            tc.tile_pool(name="agp", bufs=2) as agp,
            tc.tile_pool(name="otp", bufs=2) as otp,
            tc.tile_pool(name="wsb", bufs=3) as wsb,
            tc.tile_pool(name="psum_sc", bufs=2, space="PSUM") as psum_sc,
            tc.tile_pool(name="psum_att", bufs=1, space="PSUM") as psum_att,
            tc.tile_pool(name="psum_wk", bufs=2, space="PSUM") as psum_wk,
        ):
            # ---- persistent tiles ----
            kt_sb = [persist.tile([P, S], BF16, name=f"kt{d}") for d in range(ND)]
            v_sb = [persist.tile([P, HL, D + 1], BF16, name=f"v{t}") for t in range(NT)]
            wq_sb = persist.tile([P, ET, DG], BF16, name="wq")
            wk_sb = persist.tile([P, ET, DG], BF16, name="wk")
            wv_sb = persist.tile([P, ET, DG], BF16, name="wv")
            wo_sb = persist.tile([P, ET, EH], BF16, name="wo")
            bo_sb = persist.tile([P, EH // P], F32, name="bo")
            mask_sb = persist.tile([P, 4, SC], BF16, name="masks")
            # bc selector: row h of ones2 is 1 on cols [64h, 64h+64) -- the
            # K=2 broadcast matmul ones2.T @ [dinv0; dinv1] replicates each
            # half's reciprocal across its 64 output partitions.
            ones2_sb = persist.tile([2, P], BF16, name="ones2")

            nc.sync.dma_start(wk_sb[:], wk.rearrange("(ko p) m -> p ko m", p=P))
            nc.sync.dma_start(wq_sb[:], wq.rearrange("(ko p) m -> p ko m", p=P))
            nc.sync.dma_start(wv_sb[:], wv.rearrange("(ko p) m -> p ko m", p=P))
            nc.sync.dma_start(wo_sb[:], woT.rearrange("(ko p) m -> p ko m", p=P))
            nc.sync.dma_start(bo_sb[:], bo[:])
            nc.sync.dma_start(mask_sb[:], masks[:])
            nc.sync.dma_start(ones2_sb[:], sel[:])
            for t in range(NT):
                nc.vector.memset(v_sb[t][:, :, D], 1.0)

            cc_in = dram.tile([NSC, DG, SC], BF16)
            cc_out = dram.tile([NSC, G * DG, SC], BF16)

            xc_tiles = {}
            qt_tiles = {}
            attn_tiles = {}

            def dma_xc(c):
                xc = xp.tile([P, ET, SC], BF16, tag="xc", name=f"xc{c}")
                nc.sync.dma_start(
                    xc[:],
                    xT[:, SC * c : SC * (c + 1)].rearrange("(ko p) m -> p ko m", p=P),
                )
                xc_tiles[c] = xc

            def proj_gen(c):
                """Side-effect generator: emits K/Q/V projection ops for chunk c
                one instruction per next(), so PSUM ring allocation follows
                emission order when interleaved into the attention loop."""
                xc = xc_tiles[c]
                qt = qp.tile([P, ND, SC], BF16, tag="qt", name=f"qt{c}")
                qt_tiles[c] = qt

                def kq_tile(w_sb, d, dst_ap):
                    acc = psum_wk.tile([P, SC], F32, tag="wk", name="pacc")
                    for e in range(ET):
                        nc.tensor.matmul(
                            acc[:],
                            w_sb[:, e, P * d : P * (d + 1)],
                            xc[:, e, :],
                            start=(e == 0),
                            stop=(e == ET - 1),
                        )
                        yield
                    nc.vector.tensor_copy(dst_ap, acc[:])
                    yield

                for d in range(ND):
                    yield from kq_tile(wk_sb, d, kt_sb[d][:, SC * c : SC * (c + 1)])
                for d in range(ND):
                    yield from kq_tile(wq_sb, d, qt[:, d, :])
                for ti in range(4):
                    t = 4 * c + ti
                    acc = psum_wk.tile([P, DG], F32, tag="wk", name="pacc")
                    for e in range(ET):
                        nc.tensor.matmul(
                            acc[:],
                            xc[:, e, P * ti : P * (ti + 1)],
                            wv_sb[:, e, :],
                            start=(e == 0),
                            stop=(e == ET - 1),
                        )
                        yield
                    nc.vector.tensor_copy(
                        v_sb[t][:, :, 0:D], acc[:].rearrange("p (h d) -> p h d", d=D)
                    )
                    yield

            def outproj_gen(c):
                """Side-effect generator: output projection ops for chunk c."""
                ag = agp.tile([P, G * ND, SC], BF16, tag="ag", name=f"ag{c}")
                for k in range(G * ND):
                    nc.sync.dma_start(ag[:, k, :], cc_out[c, P * k : P * (k + 1), :])
                yield
                for et in range(EH // P):
                    acc = psum_wk.tile([P, SC], F32, tag="wk", name="oacc")
                    for k in range(G * ND):
                        nc.tensor.matmul(
                            acc[:],
                            wo_sb[:, k, P * et : P * (et + 1)],
                            ag[:, k, :],
                            start=(k == 0),
                            stop=(k == G * ND - 1),
                        )
                        yield
                    ot = otp.tile([P, SC], F32, tag="ot", name="ot")
                    nc.scalar.activation(
                        ot[:],
                        acc[:],
                        mybir.ActivationFunctionType.Identity,
                        bias=bo_sb[:, et : et + 1],
                    )
                    nc.sync.dma_start(
                        outT[P * et : P * (et + 1), SC * c : SC * (c + 1)], ot[:]
                    )
                    yield

            def attention(c, queue):
                """Attention for chunk c, draining `queue` generators between
                steps (2 deferred ops per j-step)."""
                nt = 4 * (c + 1)

                def drain(n=2):
                    emitted = 0
                    while queue and emitted < n:
                        try:
                            next(queue[0])
                            emitted += 1
                        except StopIteration:
                            queue.popleft()

                def drain_all():
                    while queue:
                        try:
                            next(queue[0])
                        except StopIteration:
                            queue.popleft()

                qt = qt_tiles[c]
                attn_t = attnp.tile([P, ND, SC], BF16, tag="attn", name=f"attn{c}")
                attn_tiles[c] = attn_t

                for d in range(ND):
                    att = psum_att.tile([D + 1, 2, SC], F32, tag="att", name="att")

                    def emit_av(j, o, ex, att=att, nt=nt):
                        for half in range(2):
                            nc.tensor.matmul(
                                att[:, half, o:SC],
                                v_sb[j][:, 2 * d + half, :],
                                ex[:, half, o:SC],
                                start=(j == 0),
                                stop=(j == nt - 1),
                            )

                    prev = None
                    for j in range(nt):
                        o = max(0, P * (j - 4 * c))  # allowed col suffix (for AV)
                        # scores full-width even on diagonal tiles: leaves no
                        # uninitialized PSUM for the full-group exp to read;
                        # the mask zeroes disallowed cols after the exp.
                        g = psum_sc.tile([P, 2, SC], F32, tag="sc", name="sco")
                        for half in range(2):
                            r = 64 * half
                            nc.tensor.matmul(
                                g[:, half, :],
                                kt_sb[d][r : r + D, P * j : P * (j + 1)],
                                qt[r : r + D, d, :],
                                start=True,
                                stop=True,
                                tile_position=(r, 0),
                            )
                        ex = expp.tile([P, 2, SC], BF16, tag="ex", name="ex")
                        nc.scalar.activation(
                            ex[:].rearrange("p a b -> p (a b)"),
                            g[:].rearrange("p a b -> p (a b)"),
                            mybir.ActivationFunctionType.Exp,
                            scale=SCALE,
                        )
                        if j >= 4 * c:  # diagonal tile: mask stale cols + wedge
                            i = j - 4 * c
                            for half in range(2):
                                nc.vector.tensor_mul(
                                    ex[:, half, :], ex[:, half, :], mask_sb[:, i, :]
                                )
                        if prev is not None:
                            emit_av(*prev)
                        drain()
                        prev = (j, o, ex)
                    emit_av(*prev)

                    # normalize: row D of att is the softmax denominator
                    bc = psum_wk.tile([P, SC], F32, tag="wk", name="bc")
                    bc_sb = wsb.tile([P, SC], F32, tag="bcs", name="bc_sb")
                    dinv1 = wsb.tile([1, 2, SC], BF16, tag="dinv1")
                    dinv2 = wsb.tile([2, SC], BF16, tag="dinv2")
                    with nc.allow_low_precision(
                        reason="bf16 reciprocal feeds broadcast matmul"
                    ):
                        nc.vector.reciprocal(dinv1[:], att[D : D + 1, :, :])
                    nc.sync.dma_start(dinv2[:], dinv1[0:1, :, :])
                    nc.tensor.matmul(
                        bc[:], ones2_sb[:], dinv2[:], start=True, stop=True
                    )
                    nc.vector.tensor_copy(bc_sb[:], bc[:])
                    for half in range(2):
                        nc.vector.tensor_mul(
                            attn_t[64 * half : 64 * half + D, d, :],
                            att[0:D, half, :],
                            bc_sb[64 * half : 64 * half + D, :],
                        )
                    nc.sync.dma_start(
                        cc_in[c, P * d : P * (d + 1), :], attn_t[:, d, :]
                    )
                    if dbg is not None:
                        nc.sync.dma_start(
                            dbg[c, P * d : P * (d + 1), :], attn_t[:, d, :]
                        )
                drain_all()
                nc.gpsimd.collective_compute(
                    "AllGather",
                    mybir.AluOpType.bypass,
                    replica_groups=[[0, 1], [2, 3], [4, 5], [6, 7]],
                    ins=[cc_in[c].opt()],
                    outs=[cc_out[c].opt()],
                )

            # ---- the fused pipeline ----
            dma_xc(0)
            for _ in proj_gen(0):
                pass
            for c in range(NSC):
                queue = deque()
                if c + 1 < NSC:
                    dma_xc(c + 1)
                    queue.append(proj_gen(c + 1))
                if c > 0:
                    queue.append(outproj_gen(c - 1))
                attention(c, queue)
            for _ in outproj_gen(NSC - 1):
                pass

    nc.compile()
    return nc


def _get_runner():
    """Build (once) and return a callable in_maps -> list of out_maps."""
    if "runner" in _CACHE:
        return _CACHE["runner"]

    nc = _build_nc()

    import jax
    from jax.sharding import Mesh, PartitionSpec
    from jax.experimental.shard_map import shard_map
    from concourse import bass2jax
    from concourse.bass2jax import _bass_exec_p, partition_id_tensor

    bass2jax.install_neuronx_cc_hook()

    in_names, out_names, out_avals, zero_shapes = [], [], [], []
    partition_name = nc.partition_id_tensor.name if nc.partition_id_tensor else None
    for alloc in nc.m.functions[0].allocations:
        if not isinstance(alloc, mybir.MemoryLocationSet):
            continue
        name = alloc.memorylocations[0].name
        if alloc.kind == "ExternalInput":
            if name != partition_name:
                in_names.append(name)
        elif alloc.kind == "ExternalOutput":
            out_names.append(name)
            shape = tuple(alloc.tensor_shape)
            dtype = mybir.dt.np(alloc.dtype)
            out_avals.append(jax.core.ShapedArray(shape, dtype))
            zero_shapes.append((shape, dtype))
    n_params = len(in_names)
    all_in_names = list(in_names) + list(out_names)
    if partition_name is not None:
        all_in_names.append(partition_name)

    def _body(*args):
        operands = list(args)
        if partition_name is not None:
            operands.append(partition_id_tensor())
        outs = _bass_exec_p.bind(
            *operands,
            out_avals=tuple(out_avals),
            in_names=tuple(all_in_names),
            out_names=tuple(out_names),
            lowering_input_output_aliases=(),
            sim_require_finite=True,
            sim_require_nnan=True,
            nc=nc,
        )
        return tuple(outs)

    devices = jax.devices()[:NCORES]
    mesh = Mesh(np.asarray(devices), ("core",))
    n_outs = len(out_names)
    sharded = jax.jit(
        shard_map(
            _body,
            mesh=mesh,
            in_specs=(PartitionSpec("core"),) * (n_params + n_outs),
            out_specs=(PartitionSpec("core"),) * n_outs,
            check_rep=False,
        ),
        donate_argnums=tuple(range(n_params, n_params + n_outs)),
        keep_unused=True,
    )

    def runner(in_maps):
        per_core = [[np.asarray(m[name]) for name in in_names] for m in in_maps]
        concat_in = [
            np.concatenate([per_core[c][i] for c in range(NCORES)], axis=0)
            for i in range(n_params)
        ]
        concat_zeros = [
            np.zeros((NCORES * s[0], *s[1:]), d) for (s, d) in zero_shapes
        ]
        out_arrs = sharded(*concat_in, *concat_zeros)
        return [
            {
                name: np.asarray(out_arrs[i]).reshape(NCORES, *out_avals[i].shape)[c]
                for i, name in enumerate(out_names)
            }
            for c in range(NCORES)
        ]

    _CACHE["runner"] = runner
    _CACHE["sharded"] = sharded
    _CACHE["mesh"] = mesh
    _CACHE["meta"] = (in_names, out_names, zero_shapes)
    return runner


def timing_setup(in_maps):
    """Device-resident timing: returns (make_zeros, call).

    `call(make_zeros())` runs one on-device execution with inputs already
    resident (zeros are donated output buffers, created outside the timer).
    """
    _get_runner()
    import jax
    from jax.sharding import NamedSharding, PartitionSpec

    in_names, out_names, zero_shapes = _CACHE["meta"]
    sharding = NamedSharding(_CACHE["mesh"], PartitionSpec("core"))
    per_core = [[np.asarray(m[name]) for name in in_names] for m in in_maps]
    dev_in = [
        jax.device_put(
            np.concatenate([per_core[c][i] for c in range(NCORES)], axis=0), sharding
        )
        for i in range(len(in_names))
    ]
    jax.block_until_ready(dev_in)

    def make_zeros():
        zs = [
            jax.device_put(np.zeros((NCORES * s[0], *s[1:]), d), sharding)
            for (s, d) in zero_shapes
        ]
        jax.block_until_ready(zs)
        return zs

    def call(zs):
        out = _CACHE["sharded"](*dev_in, *zs)
        jax.block_until_ready(out)
        return out

    return make_zeros, call


def make_in_maps(x, Wq, Wk, Wv, Wo, bo):
    """Host-side sharding: slice/transpose full inputs into per-core maps."""
    bf16 = ml_dtypes.bfloat16
    x = np.asarray(x, dtype=np.float32)
    Wq = np.asarray(Wq, dtype=np.float32)
    Wk = np.asarray(Wk, dtype=np.float32)
    Wv = np.asarray(Wv, dtype=np.float32)
    Wo = np.asarray(Wo, dtype=np.float32)
    bo = np.asarray(bo, dtype=np.float32)

    # masks[r, i, c] = 1 iff query col c >= key row r + 128*i  (diag tile i)
    r = np.arange(P)[:, None, None]
    i = np.arange(4)[None, :, None]
    cc = np.arange(SC)[None, None, :]
    masks = (cc >= r + P * i).astype(bf16)
    # bc selector: row h is 1 on cols [64h, 64h+64)
    sel = np.zeros((2, P), bf16)
    sel[0, 0:D] = 1
    sel[1, D : 2 * D] = 1

    WoT = np.ascontiguousarray(Wo.T)
    in_maps = []
    for c in range(NCORES):
        b, g = c // 2, c % 2
        xT = np.ascontiguousarray(x[b].T).astype(bf16)
        wq = np.ascontiguousarray(
            Wq[HL * g : HL * (g + 1)].transpose(1, 0, 2).reshape(E, DG)
        ).astype(bf16)
        wk = np.ascontiguousarray(
            Wk[HL * g : HL * (g + 1)].transpose(1, 0, 2).reshape(E, DG)
        ).astype(bf16)
        wv = np.ascontiguousarray(
            Wv[HL * g : HL * (g + 1)].transpose(1, 0, 2).reshape(E, DG)
        ).astype(bf16)
        woT = WoT[:, EH * g : EH * (g + 1)].astype(bf16)
        bo_c = np.ascontiguousarray(
            bo[EH * g : EH * (g + 1)].reshape(EH // P, P).T
        )
        in_maps.append(
            {
                "xT": xT,
                "wq": wq,
                "wk": wk,
                "wv": wv,
                "woT": woT,
                "bo": bo_c,
                "masks": masks,
                "sel": sel,
            }
        )
    return in_maps


def assemble_output(results):
    """Gather per-core outT [EH, S] slices into the full [B, S, E] output."""
    out = np.empty((B, S, E), dtype=np.float32)
    for c in range(NCORES):
        b, g = c // 2, c % 2
        out[b, :, EH * g : EH * (g + 1)] = results[c]["outT"].T
    return out


def kernel(x, Wq, Wk, Wv, Wo, bo):
    runner = _get_runner()
    in_maps = make_in_maps(x, Wq, Wk, Wv, Wo, bo)
    results = runner(in_maps)
    return assemble_output(results)


# revision 61
# speedup vs baseline: 239.3893x; 239.3893x over previous
"""Causal multi-head attention on 8 Trainium2 NeuronCores.

Problem: x[B=4,S=2048,E=1024], Wq/Wk/Wv[H=16,E,D=64], Wo[E,E], bo[E].
  out = softmax_causal(q k^T / sqrt(D)) v, heads concat, @ Wo.T + bo

Sharding (tensor parallel over heads, data parallel over batch):
  core c -> (batch b = c//2, head-group g = c%2 of 8 heads).
  Each core: QKV projections + attention for its 8 heads of its batch;
  pairwise AllGather (cores 2b, 2b+1) of the normalized attention outputs
  (bf16) after each sequence chunk; each core then computes half of the
  output-projection columns (e in [512g, 512g+512)) for its batch.
  Host only slices inputs / concatenates+transposes outputs.

Kernel internals (per core) -- fused chunk pipeline:
  - Everything bf16 into the PE (f32 PSUM accumulation).  Activations kept
    transposed: xT[E,S], QT/KT[dg,S], scoresT[t,s]; softmax denominator
    via a ones-column appended to V (AV matmul row 64).
  - Per sequence chunk c (512 queries): project K/V tiles and Q for chunk
    c+1 *interleaved into* chunk c's attention j-loop (a closure queue
    drains ~2 PE ops per j-step), so the PE stays busy during the
    ScalarE-exp-bound stretches and HAM keeps the PE at full clock.
  - Scores for a key tile are computed for both head-halves into one
    2-bank PSUM group [128,2,512]; a single N=1024 ACTIVATE exponentiates
    both halves (amortizes the 352-cycle ACT pipe fill).
  - Causality: query chunks of 512, key tiles of 128; diagonal tiles
    column-restricted for scores/AV and cleaned with precomputed
    [128,512] shifted-triangular masks after the (full-width) exp.
  - Out-projection of chunk c-1 and the AllGather overlap chunk c's
    attention.
"""

import os
import sys
from collections import deque

os.environ.setdefault("JAX_PLATFORMS", "axon")

for _p in ("/opt/trn_rl_repo", "/root/.axon_site/_ro/trn_rl_repo"):
    if os.path.isdir(_p) and _p not in sys.path:
        sys.path.append(_p)

import numpy as np
import ml_dtypes

import concourse.bass as bass
import concourse.mybir as mybir
import concourse.tile as tile
from concourse import bacc

B, S, E, H, D = 4, 2048, 1024, 16, 64
NCORES = 8
G = 2  # head groups
HL = H // G  # heads per core = 8
DG = HL * D  # local head dim = 512
EH = E // G  # output-projection columns per core = 512
P = 128
SC = 512  # sequence chunk
NSC = S // SC  # 4
NT = S // P  # 16 key tiles
ET = E // P  # 8 embedding tiles
ND = DG // P  # 4 head-pair tiles
SCALE = 1.0 / np.sqrt(D)

F32 = mybir.dt.float32
BF16 = mybir.dt.bfloat16

_CACHE = {}


def _build_nc():
    nc = bacc.Bacc("TRN2", target_bir_lowering=False, debug=False, num_devices=NCORES)

    xT = nc.dram_tensor("xT", [E, S], BF16, kind="ExternalInput")
    wq = nc.dram_tensor("wq", [E, DG], BF16, kind="ExternalInput")
    wk = nc.dram_tensor("wk", [E, DG], BF16, kind="ExternalInput")
    wv = nc.dram_tensor("wv", [E, DG], BF16, kind="ExternalInput")
    woT = nc.dram_tensor("woT", [E, EH], BF16, kind="ExternalInput")
    bo = nc.dram_tensor("bo", [P, EH // P], F32, kind="ExternalInput")
    masks = nc.dram_tensor("masks", [P, 4, SC], BF16, kind="ExternalInput")
    sel = nc.dram_tensor("sel", [2, P], BF16, kind="ExternalInput")
    outT = nc.dram_tensor("outT", [EH, S], F32, kind="ExternalOutput")
    dbg = None
    if os.environ.get("K_DEBUG_ATTN"):
        dbg = nc.dram_tensor("dbg", [NSC, DG, SC], BF16, kind="ExternalOutput")

    with tile.TileContext(nc) as tc:
        with (
            tc.tile_pool(name="persist", bufs=1) as persist,
            tc.tile_pool(name="dram", bufs=1, space="DRAM") as dram,
            tc.tile_pool(name="xp", bufs=2) as xp,
            tc.tile_pool(name="qp", bufs=2) as qp,
            tc.tile_pool(name="exp", bufs=4) as expp,
            tc.tile_pool(name="attn", bufs=2) as attnp,
            tc.tile_pool(name="agp", bufs=4) as agp,
            tc.tile_pool(name="otp", bufs=2) as otp,
            tc.tile_pool(name="wsb", bufs=3) as wsb,
            tc.tile_pool(name="psum_sc", bufs=2, space="PSUM") as psum_sc,
            tc.tile_pool(name="psum_att", bufs=1, space="PSUM") as psum_att,
            tc.tile_pool(name="psum_wk", bufs=2, space="PSUM") as psum_wk,
        ):
            # ---- persistent tiles ----
            kt_sb = [persist.tile([P, S], BF16, name=f"kt{d}") for d in range(ND)]
            v_sb = [persist.tile([P, HL, D + 1], BF16, name=f"v{t}") for t in range(NT)]
            wq_sb = persist.tile([P, ET, DG], BF16, name="wq")
            wk_sb = persist.tile([P, ET, DG], BF16, name="wk")
            wv_sb = persist.tile([P, ET, DG], BF16, name="wv")
            wo_sb = persist.tile([P, ET, EH], BF16, name="wo")
            bo_sb = persist.tile([P, EH // P], F32, name="bo")
            mask_sb = persist.tile([P, 4, SC], BF16, name="masks")
            # bc selector: row h of ones2 is 1 on cols [64h, 64h+64) -- the
            # K=2 broadcast matmul ones2.T @ [dinv0; dinv1] replicates each
            # half's reciprocal across its 64 output partitions.
            ones2_sb = persist.tile([2, P], BF16, name="ones2")

            # wq + x chunk 0 gate the first projection matmuls -- load them
            # first, on different engine DMA queues so they transfer in
            # parallel; the rest follows in need-order.
            # first Q-proj matmuls need wq e-tiles 0.. and xc0 e-tiles 0..
            # in order -- split each across two queues at the E midpoint so
            # the low halves land first, in parallel.
            xc0 = xp.tile([P, ET, SC], BF16, tag="xc", name="xc0")
            xre = "(ko p) m -> p ko m"
            nc.scalar.dma_start(xc0[:, 0:4, :], xT[0:512, 0:SC].rearrange(xre, p=P))
            nc.sync.dma_start(wq_sb[:, 0:4, :], wq[0:512, :].rearrange(xre, p=P))
            nc.sync.dma_start(xc0[:, 4:8, :], xT[512:E, 0:SC].rearrange(xre, p=P))
            nc.scalar.dma_start(wq_sb[:, 4:8, :], wq[512:E, :].rearrange(xre, p=P))
            nc.gpsimd.dma_start(wk_sb[:], wk.rearrange(xre, p=P))
            nc.gpsimd.dma_start(wv_sb[:], wv.rearrange(xre, p=P))
            nc.gpsimd.dma_start(wo_sb[:], woT.rearrange(xre, p=P))
            nc.gpsimd.dma_start(bo_sb[:], bo[:])
            nc.sync.dma_start(mask_sb[:], masks[:])
            nc.sync.dma_start(ones2_sb[:], sel[:])
            for t in range(NT):
                nc.vector.memset(v_sb[t][:, :, D], 1.0)
            for _ in range(2):  # zero the score ring so the first full-group
                g0 = psum_sc.tile([P, 2, SC], F32, tag="sc", name="scz")
                nc.vector.memset(g0[:], 0.0)  # exps read finite stale cols

            # per-(chunk, d) AllGather: gather d's 128 dims right after its
            # normalization, so the out-projection's accumulation over k can
            # follow the gathers and the final tail only waits on the last
            # d's small gather.  Gather d rows = both ranks' dims [128d,
            # 128d+128) = Wo row-tiles {d, 4+d}.
            cc_in = dram.tile([NSC, DG, SC], BF16)
            cc_outd = dram.tile([NSC, ND, 2 * P, SC], BF16)

            xc_tiles = {0: xc0}
            qt_tiles = {}
            attn_tiles = {}

            def dma_xc(c):
                xc = xp.tile([P, ET, SC], BF16, tag="xc", name=f"xc{c}")
                nc.sync.dma_start(
                    xc[:],
                    xT[:, SC * c : SC * (c + 1)].rearrange("(ko p) m -> p ko m", p=P),
                )
                xc_tiles[c] = xc

            def proj_gen(c, skip_q=False):
                """Side-effect generator: emits K/Q/V projection ops for chunk c
                one instruction per next(), so PSUM ring allocation follows
                emission order when interleaved into the attention loop."""
                xc = xc_tiles[c]
                if not skip_q:
                    qt = qp.tile([P, ND, SC], BF16, tag="qt", name=f"qt{c}")
                    qt_tiles[c] = qt

                def kq_tile(w_sb, d, dst_ap):
                    acc = psum_wk.tile([P, SC], F32, tag="wk", name="pacc")
                    for e in range(ET):
                        nc.tensor.matmul(
                            acc[:],
                            w_sb[:, e, P * d : P * (d + 1)],
                            xc[:, e, :],
                            start=(e == 0),
                            stop=(e == ET - 1),
                        )
                        yield
                    nc.vector.tensor_copy(dst_ap, acc[:])
                    yield

                # Q first: attention chunk c needs all of qt(c) at its first
                # j-step, but kt/v of chunk c only at j = 4c (late).
                if not skip_q:
                    for d in range(ND):
                        yield from kq_tile(wq_sb, d, qt[:, d, :])
                for d in range(ND):
                    yield from kq_tile(wk_sb, d, kt_sb[d][:, SC * c : SC * (c + 1)])
                for ti in range(4):
                    t = 4 * c + ti
                    acc = psum_wk.tile([P, DG], F32, tag="wk", name="pacc")
                    for e in range(ET):
                        nc.tensor.matmul(
                            acc[:],
                            xc[:, e, P * ti : P * (ti + 1)],
                            wv_sb[:, e, :],
                            start=(e == 0),
                            stop=(e == ET - 1),
                        )
                        yield
                    nc.vector.tensor_copy(
                        v_sb[t][:, :, 0:D], acc[:].rearrange("p (h d) -> p h d", d=D)
                    )
                    yield

            def outproj_gen(c):
                """Side-effect generator: output projection ops for chunk c.
                ag DMAs are triggered from the GpSimd queue (where the
                collectives run) so they never head-of-line-block the Sync
                queue's critical-path DMAs; per-et accumulation follows the
                per-d gather order so only the last gather gates the tail."""
                ag = agp.tile([P, 2 * ND, SC], BF16, tag="ag", name=f"ag{c}")
                for d4 in range(ND):
                    nc.gpsimd.dma_start(
                        ag[:, 2 * d4, :], cc_outd[c, d4, 0:P, :]
                    )
                    nc.gpsimd.dma_start(
                        ag[:, 2 * d4 + 1, :], cc_outd[c, d4, P : 2 * P, :]
                    )
                    yield
                for et in range(EH // P):
                    acc = psum_wk.tile([P, SC], F32, tag="wk", name="oacc")
                    for d4 in range(ND):
                        for ri in range(2):  # rank 0 / rank 1 half
                            nc.tensor.matmul(
                                acc[:],
                                wo_sb[:, 4 * ri + d4, P * et : P * (et + 1)],
                                ag[:, 2 * d4 + ri, :],
                                start=(d4 == 0 and ri == 0),
                                stop=(d4 == ND - 1 and ri == 1),
                            )
                            yield
                    ot = otp.tile([P, SC], F32, tag="ot", name="ot")
                    nc.vector.tensor_scalar_add(ot[:], acc[:], bo_sb[:, et : et + 1])
                    nc.sync.dma_start(
                        outT[P * et : P * (et + 1), SC * c : SC * (c + 1)], ot[:]
                    )
                    yield

            def attention(c, queue):
                """Attention for chunk c, draining `queue` generators between
                steps (2 deferred ops per j-step).  In the last chunk the
                final ~15 steps stop draining so the leftovers run during the
                final collective's flight (fills the tail)."""
                nt = 4 * (c + 1)
                steps = [0]
                reserve_after = ND * nt - 15 if c == NSC - 1 else 1 << 30

                def drain(n=2):
                    steps[0] += 1
                    if steps[0] > reserve_after:
                        return
                    emitted = 0
                    while queue and emitted < n:
                        try:
                            next(queue[0])
                            emitted += 1
                        except StopIteration:
                            queue.popleft()

                def drain_all():
                    while queue:
                        try:
                            next(queue[0])
                        except StopIteration:
                            queue.popleft()

                qt = qt_tiles[c]
                attn_t = attnp.tile([P, ND, SC], BF16, tag="attn", name=f"attn{c}")
                attn_tiles[c] = attn_t

                for d in range(ND):
                    att = psum_att.tile([D + 1, 2, SC], F32, tag="att", name="att")

                    def emit_av(j, o, ex, att=att, nt=nt):
                        for half in range(2):
                            nc.tensor.matmul(
                                att[:, half, o:SC],
                                v_sb[j][:, 2 * d + half, :],
                                ex[:, half, o:SC],
                                start=(j == 0),
                                stop=(j == nt - 1),
                            )

                    prev = None
                    for j in range(nt):
                        o = max(0, P * (j - 4 * c))  # allowed col suffix (for AV)
                        # diagonal scores column-restricted; the exp still
                        # reads the full group (stale cols hold the memset
                        # zeros or finite old scores) and the mask zeroes
                        # the disallowed columns afterwards.
                        g = psum_sc.tile([P, 2, SC], F32, tag="sc", name="sco")
                        for half in range(2):
                            r = 64 * half
                            nc.tensor.matmul(
                                g[:, half, o:SC],
                                kt_sb[d][r : r + D, P * j : P * (j + 1)],
                                qt[r : r + D, d, o:SC],
                                start=True,
                                stop=True,
                                tile_position=(r, 0),
                            )
                        ex = expp.tile([P, 2, SC], BF16, tag="ex", name="ex")
                        nc.scalar.activation(
                            ex[:].rearrange("p a b -> p (a b)"),
                            g[:].rearrange("p a b -> p (a b)"),
                            mybir.ActivationFunctionType.Exp,
                            scale=SCALE,
                        )
                        if j >= 4 * c:  # diagonal tile: mask stale cols + wedge
                            i = j - 4 * c
                            for half in range(2):
                                nc.vector.tensor_mul(
                                    ex[:, half, :], ex[:, half, :], mask_sb[:, i, :]
                                )
                        if prev is not None:
                            emit_av(*prev)
                        drain()
                        prev = (j, o, ex)
                    emit_av(*prev)

                    # normalize: row D of att is the softmax denominator.
                    # Broadcast den to 128 partitions via the selector matmul
                    # FIRST, then a 128-lane approx reciprocal -- a 1-lane DVE
                    # reciprocal is serial (~8 us) and was the v2 bottleneck.
                    bc = psum_wk.tile([P, SC], F32, tag="wk", name="bc")
                    bc_sb = wsb.tile([P, SC], F32, tag="bcs", name="bc_sb")
                    den1 = wsb.tile([1, 2, SC], BF16, tag="den1")
                    den2 = wsb.tile([2, SC], BF16, tag="den2")
                    nc.vector.tensor_copy(den1[:], att[D : D + 1, :, :])
                    nc.sync.dma_start(den2[:], den1[0:1, :, :])
                    nc.tensor.matmul(
                        bc[:], ones2_sb[:], den2[:], start=True, stop=True
                    )
                    nc.vector.reciprocal_approx_fast(bc_sb[:], bc[:])
                    for half in range(2):
                        nc.vector.tensor_mul(
                            attn_t[64 * half : 64 * half + D, d, :],
                            att[0:D, half, :],
                            bc_sb[64 * half : 64 * half + D, :],
                        )
                    nc.sync.dma_start(
                        cc_in[c, P * d : P * (d + 1), :], attn_t[:, d, :]
                    )
                    if dbg is not None:
                        nc.sync.dma_start(
                            dbg[c, P * d : P * (d + 1), :], attn_t[:, d, :]
                        )
                    nc.gpsimd.collective_compute(
                        "AllGather",
                        mybir.AluOpType.bypass,
                        replica_groups=[[0, 1], [2, 3], [4, 5], [6, 7]],
                        ins=[cc_in[c, P * d : P * (d + 1)].opt()],
                        outs=[cc_outd[c, d].opt()],
                    )
                drain_all()

            # ---- the fused pipeline ----  (xc0 DMA'd in the preamble)
            # prologue Q-projection consumes e-tiles in DMA-arrival order:
            # low halves of d0/d1 run while the high halves land.
            qt0 = qp.tile([P, ND, SC], BF16, tag="qt", name="qt0")
            qt_tiles[0] = qt0
            _accs = {}

            def _qlo(d):
                acc = psum_wk.tile([P, SC], F32, tag="wk", name="pacc")
                _accs[d] = acc
                for e in range(4):
                    nc.tensor.matmul(
                        acc[:],
                        wq_sb[:, e, P * d : P * (d + 1)],
                        xc0[:, e, :],
                        start=(e == 0),
                        stop=False,
                    )

            def _qhi(d):
                acc = _accs.pop(d)
                for e in range(4, ET):
                    nc.tensor.matmul(
                        acc[:],
                        wq_sb[:, e, P * d : P * (d + 1)],
                        xc0[:, e, :],
                        start=False,
                        stop=(e == ET - 1),
                    )
                nc.vector.tensor_copy(qt0[:, d, :], acc[:])

            for op in (lambda: _qlo(0), lambda: _qlo(1), lambda: _qhi(0),
                       lambda: _qlo(2), lambda: _qhi(1), lambda: _qlo(3),
                       lambda: _qhi(2), lambda: _qhi(3)):
                op()
            for _ in proj_gen(0, skip_q=True):
                pass
            # chunk-3's attention has the most idle PE slots (longest j-loops,
            # no projection work left), so defer outproj(1) there.
            for c in range(NSC):
                queue = deque()
                if c + 1 < NSC:
                    dma_xc(c + 1)
                    queue.append(proj_gen(c + 1))
                if c == 1:
                    queue.append(outproj_gen(0))
                elif c == 3:
                    queue.append(outproj_gen(1))
                    queue.append(outproj_gen(2))
                attention(c, queue)
            for _ in outproj_gen(NSC - 1):
                pass

    nc.compile()
    return nc


def _build_once():
    if "nc" not in _CACHE:
        _CACHE["nc"] = _build_nc()
    return _CACHE["nc"]


def _get_runner():
    """Build (once) and return a callable in_maps -> list of out_maps."""
    if "runner" in _CACHE:
        return _CACHE["runner"]

    nc = _build_once()

    import jax
    from jax.sharding import Mesh, PartitionSpec
    from jax.experimental.shard_map import shard_map
    from concourse import bass2jax
    from concourse.bass2jax import _bass_exec_p, partition_id_tensor

    bass2jax.install_neuronx_cc_hook()

    in_names, out_names, out_avals, zero_shapes = [], [], [], []
    partition_name = nc.partition_id_tensor.name if nc.partition_id_tensor else None
    for alloc in nc.m.functions[0].allocations:
        if not isinstance(alloc, mybir.MemoryLocationSet):
            continue
        name = alloc.memorylocations[0].name
        if alloc.kind == "ExternalInput":
            if name != partition_name:
                in_names.append(name)
        elif alloc.kind == "ExternalOutput":
            out_names.append(name)
            shape = tuple(alloc.tensor_shape)
            dtype = mybir.dt.np(alloc.dtype)
            out_avals.append(jax.core.ShapedArray(shape, dtype))
            zero_shapes.append((shape, dtype))
    n_params = len(in_names)
    all_in_names = list(in_names) + list(out_names)
    if partition_name is not None:
        all_in_names.append(partition_name)

    def _body(*args):
        operands = list(args)
        if partition_name is not None:
            operands.append(partition_id_tensor())
        outs = _bass_exec_p.bind(
            *operands,
            out_avals=tuple(out_avals),
            in_names=tuple(all_in_names),
            out_names=tuple(out_names),
            lowering_input_output_aliases=(),
            sim_require_finite=True,
            sim_require_nnan=True,
            nc=nc,
        )
        return tuple(outs)

    devices = jax.devices()[:NCORES]
    mesh = Mesh(np.asarray(devices), ("core",))
    n_outs = len(out_names)
    sharded = jax.jit(
        shard_map(
            _body,
            mesh=mesh,
            in_specs=(PartitionSpec("core"),) * (n_params + n_outs),
            out_specs=(PartitionSpec("core"),) * n_outs,
            check_rep=False,
        ),
        donate_argnums=tuple(range(n_params, n_params + n_outs)),
        keep_unused=True,
    )

    def runner(in_maps):
        per_core = [[np.asarray(m[name]) for name in in_names] for m in in_maps]
        concat_in = [
            np.concatenate([per_core[c][i] for c in range(NCORES)], axis=0)
            for i in range(n_params)
        ]
        concat_zeros = [
            np.zeros((NCORES * s[0], *s[1:]), d) for (s, d) in zero_shapes
        ]
        out_arrs = sharded(*concat_in, *concat_zeros)
        return [
            {
                name: np.asarray(out_arrs[i]).reshape(NCORES, *out_avals[i].shape)[c]
                for i, name in enumerate(out_names)
            }
            for c in range(NCORES)
        ]

    _CACHE["runner"] = runner
    _CACHE["sharded"] = sharded
    _CACHE["mesh"] = mesh
    _CACHE["meta"] = (in_names, out_names, zero_shapes)
    return runner


def timing_setup(in_maps):
    """Device-resident timing: returns (make_zeros, call).

    `call(make_zeros())` runs one on-device execution with inputs already
    resident (zeros are donated output buffers, created outside the timer).
    """
    _get_runner()
    import jax
    from jax.sharding import NamedSharding, PartitionSpec

    in_names, out_names, zero_shapes = _CACHE["meta"]
    sharding = NamedSharding(_CACHE["mesh"], PartitionSpec("core"))
    per_core = [[np.asarray(m[name]) for name in in_names] for m in in_maps]
    dev_in = [
        jax.device_put(
            np.concatenate([per_core[c][i] for c in range(NCORES)], axis=0), sharding
        )
        for i in range(len(in_names))
    ]
    jax.block_until_ready(dev_in)

    def make_zeros():
        zs = [
            jax.device_put(np.zeros((NCORES * s[0], *s[1:]), d), sharding)
            for (s, d) in zero_shapes
        ]
        jax.block_until_ready(zs)
        return zs

    def call(zs):
        out = _CACHE["sharded"](*dev_in, *zs)
        jax.block_until_ready(out)
        return out

    return make_zeros, call


def make_in_maps(x, Wq, Wk, Wv, Wo, bo):
    """Host-side sharding: slice/transpose full inputs into per-core maps."""
    bf16 = ml_dtypes.bfloat16
    x = np.asarray(x, dtype=np.float32)
    Wq = np.asarray(Wq, dtype=np.float32)
    Wk = np.asarray(Wk, dtype=np.float32)
    Wv = np.asarray(Wv, dtype=np.float32)
    Wo = np.asarray(Wo, dtype=np.float32)
    bo = np.asarray(bo, dtype=np.float32)

    # masks[r, i, c] = 1 iff query col c >= key row r + 128*i  (diag tile i)
    r = np.arange(P)[:, None, None]
    i = np.arange(4)[None, :, None]
    cc = np.arange(SC)[None, None, :]
    masks = (cc >= r + P * i).astype(bf16)
    # bc selector: row h is 1 on cols [64h, 64h+64)
    sel = np.zeros((2, P), bf16)
    sel[0, 0:D] = 1
    sel[1, D : 2 * D] = 1

    WoT = np.ascontiguousarray(Wo.T)
    in_maps = []
    for c in range(NCORES):
        b, g = c // 2, c % 2
        xT = np.ascontiguousarray(x[b].T).astype(bf16)
        wq = np.ascontiguousarray(
            Wq[HL * g : HL * (g + 1)].transpose(1, 0, 2).reshape(E, DG)
        ).astype(bf16)
        wk = np.ascontiguousarray(
            Wk[HL * g : HL * (g + 1)].transpose(1, 0, 2).reshape(E, DG)
        ).astype(bf16)
        wv = np.ascontiguousarray(
            Wv[HL * g : HL * (g + 1)].transpose(1, 0, 2).reshape(E, DG)
        ).astype(bf16)
        woT = WoT[:, EH * g : EH * (g + 1)].astype(bf16)
        bo_c = np.ascontiguousarray(
            bo[EH * g : EH * (g + 1)].reshape(EH // P, P).T
        )
        in_maps.append(
            {
                "xT": xT,
                "wq": wq,
                "wk": wk,
                "wv": wv,
                "woT": woT,
                "bo": bo_c,
                "masks": masks,
                "sel": sel,
            }
        )
    return in_maps


def assemble_output(results):
    """Gather per-core outT [EH, S] slices into the full [B, S, E] output."""
    out = np.empty((B, S, E), dtype=np.float32)
    for c in range(NCORES):
        b, g = c // 2, c % 2
        out[b, :, EH * g : EH * (g + 1)] = results[c]["outT"].T
    return out


def kernel(x, Wq, Wk, Wv, Wo, bo):
    runner = _get_runner()
    in_maps = make_in_maps(x, Wq, Wk, Wv, Wo, bo)
    results = runner(in_maps)
    return assemble_output(results)


# revision 62
# speedup vs baseline: 248.2406x; 1.0370x over previous
"""Causal multi-head attention on 8 Trainium2 NeuronCores.

Problem: x[B=4,S=2048,E=1024], Wq/Wk/Wv[H=16,E,D=64], Wo[E,E], bo[E].
  out = softmax_causal(q k^T / sqrt(D)) v, heads concat, @ Wo.T + bo

Sharding (tensor parallel over heads, data parallel over batch):
  core c -> (batch b = c//2, head-group g = c%2 of 8 heads).
  Each core: QKV projections + attention for its 8 heads of its batch;
  pairwise AllGather (cores 2b, 2b+1) of the normalized attention outputs
  (bf16) after each sequence chunk; each core then computes half of the
  output-projection columns (e in [512g, 512g+512)) for its batch.
  Host only slices inputs / concatenates+transposes outputs.

Kernel internals (per core) -- fused chunk pipeline:
  - Everything bf16 into the PE (f32 PSUM accumulation).  Activations kept
    transposed: xT[E,S], QT/KT[dg,S], scoresT[t,s]; softmax denominator
    via a ones-column appended to V (AV matmul row 64).
  - Per sequence chunk c (512 queries): project K/V tiles and Q for chunk
    c+1 *interleaved into* chunk c's attention j-loop (a closure queue
    drains ~2 PE ops per j-step), so the PE stays busy during the
    ScalarE-exp-bound stretches and HAM keeps the PE at full clock.
  - Scores for a key tile are computed for both head-halves into one
    2-bank PSUM group [128,2,512]; a single N=1024 ACTIVATE exponentiates
    both halves (amortizes the 352-cycle ACT pipe fill).
  - Causality: query chunks of 512, key tiles of 128; diagonal tiles
    column-restricted for scores/AV and cleaned with precomputed
    [128,512] shifted-triangular masks after the (full-width) exp.
  - Out-projection of chunk c-1 and the AllGather overlap chunk c's
    attention.
"""

import os
import sys
from collections import deque

os.environ.setdefault("JAX_PLATFORMS", "axon")

for _p in ("/opt/trn_rl_repo", "/root/.axon_site/_ro/trn_rl_repo"):
    if os.path.isdir(_p) and _p not in sys.path:
        sys.path.append(_p)

import numpy as np
import ml_dtypes

import concourse.bass as bass
import concourse.mybir as mybir
import concourse.tile as tile
from concourse import bacc

B, S, E, H, D = 4, 2048, 1024, 16, 64
NCORES = 8
G = 2  # head groups
HL = H // G  # heads per core = 8
DG = HL * D  # local head dim = 512
EH = E // G  # output-projection columns per core = 512
P = 128
SC = 512  # sequence chunk
NSC = S // SC  # 4
NT = S // P  # 16 key tiles
ET = E // P  # 8 embedding tiles
ND = DG // P  # 4 head-pair tiles
SCALE = 1.0 / np.sqrt(D)

F32 = mybir.dt.float32
BF16 = mybir.dt.bfloat16

_CACHE = {}


def _build_nc():
    nc = bacc.Bacc("TRN2", target_bir_lowering=False, debug=False, num_devices=NCORES)

    xT = nc.dram_tensor("xT", [E, S], BF16, kind="ExternalInput")
    wq = nc.dram_tensor("wq", [E, DG], BF16, kind="ExternalInput")
    wk = nc.dram_tensor("wk", [E, DG], BF16, kind="ExternalInput")
    wv = nc.dram_tensor("wv", [E, DG], BF16, kind="ExternalInput")
    woT = nc.dram_tensor("woT", [E, EH], BF16, kind="ExternalInput")
    bo = nc.dram_tensor("bo", [P, EH // P], F32, kind="ExternalInput")
    masks = nc.dram_tensor("masks", [P, 4, SC], BF16, kind="ExternalInput")
    sel = nc.dram_tensor("sel", [2, P], BF16, kind="ExternalInput")
    outT = nc.dram_tensor("outT", [EH, S], F32, kind="ExternalOutput")
    dbg = None
    if os.environ.get("K_DEBUG_ATTN"):
        dbg = nc.dram_tensor("dbg", [NSC, DG, SC], BF16, kind="ExternalOutput")

    with tile.TileContext(nc) as tc:
        with (
            tc.tile_pool(name="persist", bufs=1) as persist,
            tc.tile_pool(name="dram", bufs=1, space="DRAM") as dram,
            tc.tile_pool(name="xp", bufs=2) as xp,
            tc.tile_pool(name="qp", bufs=2) as qp,
            tc.tile_pool(name="exp", bufs=5) as expp,
            tc.tile_pool(name="attn", bufs=2) as attnp,
            tc.tile_pool(name="agp", bufs=4) as agp,
            tc.tile_pool(name="otp", bufs=2) as otp,
            tc.tile_pool(name="wsb", bufs=3) as wsb,
            tc.tile_pool(name="psum_sc", bufs=2, space="PSUM") as psum_sc,
            tc.tile_pool(name="psum_att", bufs=1, space="PSUM") as psum_att,
            tc.tile_pool(name="psum_wk", bufs=2, space="PSUM") as psum_wk,
        ):
            # ---- persistent tiles ----
            kt_sb = [persist.tile([P, S], BF16, name=f"kt{d}") for d in range(ND)]
            v_sb = [persist.tile([P, HL, D + 1], BF16, name=f"v{t}") for t in range(NT)]
            wq_sb = persist.tile([P, ET, DG], BF16, name="wq")
            wk_sb = persist.tile([P, ET, DG], BF16, name="wk")
            wv_sb = persist.tile([P, ET, DG], BF16, name="wv")
            wo_sb = persist.tile([P, ET, EH], BF16, name="wo")
            bo_sb = persist.tile([P, EH // P], F32, name="bo")
            mask_sb = persist.tile([P, 4, SC], BF16, name="masks")
            # bc selector: row h of ones2 is 1 on cols [64h, 64h+64) -- the
            # K=2 broadcast matmul ones2.T @ [dinv0; dinv1] replicates each
            # half's reciprocal across its 64 output partitions.
            ones2_sb = persist.tile([2, P], BF16, name="ones2")

            # wq + x chunk 0 gate the first projection matmuls -- load them
            # first, on different engine DMA queues so they transfer in
            # parallel; the rest follows in need-order.
            # first Q-proj matmuls need wq e-tiles 0.. and xc0 e-tiles 0..
            # in order -- split each across two queues at the E midpoint so
            # the low halves land first, in parallel.
            xc0 = xp.tile([P, ET, SC], BF16, tag="xc", name="xc0")
            xre = "(ko p) m -> p ko m"
            nc.scalar.dma_start(xc0[:, 0:4, :], xT[0:512, 0:SC].rearrange(xre, p=P))
            nc.sync.dma_start(wq_sb[:, 0:4, :], wq[0:512, :].rearrange(xre, p=P))
            nc.sync.dma_start(xc0[:, 4:8, :], xT[512:E, 0:SC].rearrange(xre, p=P))
            nc.scalar.dma_start(wq_sb[:, 4:8, :], wq[512:E, :].rearrange(xre, p=P))
            nc.gpsimd.dma_start(wk_sb[:], wk.rearrange(xre, p=P))
            nc.gpsimd.dma_start(wv_sb[:], wv.rearrange(xre, p=P))
            nc.gpsimd.dma_start(wo_sb[:], woT.rearrange(xre, p=P))
            nc.gpsimd.dma_start(bo_sb[:], bo[:])
            nc.sync.dma_start(mask_sb[:], masks[:])
            nc.sync.dma_start(ones2_sb[:], sel[:])
            for t in range(NT):
                nc.vector.memset(v_sb[t][:, :, D], 1.0)
            for _ in range(2):  # zero the score ring so the first full-group
                g0 = psum_sc.tile([P, 2, SC], F32, tag="sc", name="scz")
                nc.vector.memset(g0[:], 0.0)  # exps read finite stale cols

            # per-(chunk, d) AllGather: gather d's 128 dims right after its
            # normalization, so the out-projection's accumulation over k can
            # follow the gathers and the final tail only waits on the last
            # d's small gather.  Gather d rows = both ranks' dims [128d,
            # 128d+128) = Wo row-tiles {d, 4+d}.
            cc_in = dram.tile([NSC, DG, SC], BF16)
            cc_outd = dram.tile([NSC, ND, 2 * P, SC], BF16)

            xc_tiles = {0: xc0}
            qt_tiles = {}
            attn_tiles = {}

            def dma_xc(c):
                xc = xp.tile([P, ET, SC], BF16, tag="xc", name=f"xc{c}")
                nc.sync.dma_start(
                    xc[:],
                    xT[:, SC * c : SC * (c + 1)].rearrange("(ko p) m -> p ko m", p=P),
                )
                xc_tiles[c] = xc

            def proj_gen(c, skip_q=False):
                """Side-effect generator: emits K/Q/V projection ops for chunk c
                one instruction per next(), so PSUM ring allocation follows
                emission order when interleaved into the attention loop."""
                xc = xc_tiles[c]
                if not skip_q:
                    qt = qp.tile([P, ND, SC], BF16, tag="qt", name=f"qt{c}")
                    qt_tiles[c] = qt

                def kq_tile(w_sb, d, dst_ap):
                    acc = psum_wk.tile([P, SC], F32, tag="wk", name="pacc")
                    for e in range(ET):
                        nc.tensor.matmul(
                            acc[:],
                            w_sb[:, e, P * d : P * (d + 1)],
                            xc[:, e, :],
                            start=(e == 0),
                            stop=(e == ET - 1),
                        )
                        yield
                    nc.vector.tensor_copy(dst_ap, acc[:])
                    yield

                # Q first: attention chunk c needs all of qt(c) at its first
                # j-step, but kt/v of chunk c only at j = 4c (late).
                if not skip_q:
                    for d in range(ND):
                        yield from kq_tile(wq_sb, d, qt[:, d, :])
                for d in range(ND):
                    yield from kq_tile(wk_sb, d, kt_sb[d][:, SC * c : SC * (c + 1)])
                for ti in range(4):
                    t = 4 * c + ti
                    acc = psum_wk.tile([P, DG], F32, tag="wk", name="pacc")
                    for e in range(ET):
                        nc.tensor.matmul(
                            acc[:],
                            xc[:, e, P * ti : P * (ti + 1)],
                            wv_sb[:, e, :],
                            start=(e == 0),
                            stop=(e == ET - 1),
                        )
                        yield
                    nc.vector.tensor_copy(
                        v_sb[t][:, :, 0:D], acc[:].rearrange("p (h d) -> p h d", d=D)
                    )
                    yield

            def outproj_gen(c):
                """Side-effect generator: output projection ops for chunk c.
                ag DMAs are triggered from the GpSimd queue (where the
                collectives run) so they never head-of-line-block the Sync
                queue's critical-path DMAs; per-et accumulation follows the
                per-d gather order so only the last gather gates the tail."""
                ag = agp.tile([P, 2 * ND, SC], BF16, tag="ag", name=f"ag{c}")
                for d4 in range(ND):
                    nc.gpsimd.dma_start(
                        ag[:, 2 * d4, :], cc_outd[c, d4, 0:P, :]
                    )
                    nc.gpsimd.dma_start(
                        ag[:, 2 * d4 + 1, :], cc_outd[c, d4, P : 2 * P, :]
                    )
                    yield
                for et in range(EH // P):
                    acc = psum_wk.tile([P, SC], F32, tag="wk", name="oacc")
                    for d4 in range(ND):
                        for ri in range(2):  # rank 0 / rank 1 half
                            nc.tensor.matmul(
                                acc[:],
                                wo_sb[:, 4 * ri + d4, P * et : P * (et + 1)],
                                ag[:, 2 * d4 + ri, :],
                                start=(d4 == 0 and ri == 0),
                                stop=(d4 == ND - 1 and ri == 1),
                            )
                            yield
                    ot = otp.tile([P, SC], F32, tag="ot", name="ot")
                    nc.vector.tensor_scalar_add(ot[:], acc[:], bo_sb[:, et : et + 1])
                    nc.sync.dma_start(
                        outT[P * et : P * (et + 1), SC * c : SC * (c + 1)], ot[:]
                    )
                    yield

            def attention(c, queue):
                """Attention for chunk c, draining `queue` generators between
                steps (2 deferred ops per j-step).  In the last chunk the
                final ~15 steps stop draining so the leftovers run during the
                final collective's flight (fills the tail)."""
                nt = 4 * (c + 1)
                steps = [0]
                reserve_after = ND * nt - 15 if c == NSC - 1 else 1 << 30

                def drain(n=1, count_step=False):
                    if count_step:
                        steps[0] += 1
                    if steps[0] > reserve_after:
                        return
                    emitted = 0
                    while queue and emitted < n:
                        try:
                            next(queue[0])
                            emitted += 1
                        except StopIteration:
                            queue.popleft()

                def drain_all():
                    while queue:
                        try:
                            next(queue[0])
                        except StopIteration:
                            queue.popleft()

                qt = qt_tiles[c]
                attn_t = attnp.tile([P, ND, SC], BF16, tag="attn", name=f"attn{c}")
                attn_tiles[c] = attn_t

                for d in range(ND):
                    att = psum_att.tile([D + 1, 2, SC], F32, tag="att", name="att")

                    def emit_av(j, o, ex, att=att, nt=nt):
                        for half in range(2):
                            nc.tensor.matmul(
                                att[:, half, o:SC],
                                v_sb[j][:, 2 * d + half, :],
                                ex[:, half, o:SC],
                                start=(j == 0),
                                stop=(j == nt - 1),
                            )

                    prev = None
                    for j in range(nt):
                        o = max(0, P * (j - 4 * c))  # allowed col suffix (for AV)
                        # diagonal scores column-restricted; the exp still
                        # reads the full group (stale cols hold the memset
                        # zeros or finite old scores) and the mask zeroes
                        # the disallowed columns afterwards.
                        g = psum_sc.tile([P, 2, SC], F32, tag="sc", name="sco")
                        for half in range(2):
                            r = 64 * half
                            nc.tensor.matmul(
                                g[:, half, o:SC],
                                kt_sb[d][r : r + D, P * j : P * (j + 1)],
                                qt[r : r + D, d, o:SC],
                                start=True,
                                stop=True,
                                tile_position=(r, 0),
                            )
                        drain(1, count_step=True)
                        ex = expp.tile([P, 2, SC], BF16, tag="ex", name="ex")
                        nc.scalar.activation(
                            ex[:].rearrange("p a b -> p (a b)"),
                            g[:].rearrange("p a b -> p (a b)"),
                            mybir.ActivationFunctionType.Exp,
                            scale=SCALE,
                        )
                        if j >= 4 * c:  # diagonal tile: mask stale cols + wedge
                            i = j - 4 * c
                            for half in range(2):
                                nc.vector.tensor_mul(
                                    ex[:, half, :], ex[:, half, :], mask_sb[:, i, :]
                                )
                        if prev is not None:
                            emit_av(*prev)
                        drain(1)
                        prev = (j, o, ex)
                    emit_av(*prev)

                    # normalize: row D of att is the softmax denominator.
                    # Broadcast den to 128 partitions via the selector matmul
                    # FIRST, then a 128-lane approx reciprocal -- a 1-lane DVE
                    # reciprocal is serial (~8 us) and was the v2 bottleneck.
                    bc = psum_wk.tile([P, SC], F32, tag="wk", name="bc")
                    bc_sb = wsb.tile([P, SC], F32, tag="bcs", name="bc_sb")
                    den1 = wsb.tile([1, 2, SC], BF16, tag="den1")
                    den2 = wsb.tile([2, SC], BF16, tag="den2")
                    nc.vector.tensor_copy(den1[:], att[D : D + 1, :, :])
                    nc.sync.dma_start(den2[:], den1[0:1, :, :])
                    nc.tensor.matmul(
                        bc[:], ones2_sb[:], den2[:], start=True, stop=True
                    )
                    nc.vector.reciprocal_approx_fast(bc_sb[:], bc[:])
                    for half in range(2):
                        nc.vector.tensor_mul(
                            attn_t[64 * half : 64 * half + D, d, :],
                            att[0:D, half, :],
                            bc_sb[64 * half : 64 * half + D, :],
                        )
                    nc.sync.dma_start(
                        cc_in[c, P * d : P * (d + 1), :], attn_t[:, d, :]
                    )
                    if dbg is not None:
                        nc.sync.dma_start(
                            dbg[c, P * d : P * (d + 1), :], attn_t[:, d, :]
                        )
                    nc.gpsimd.collective_compute(
                        "AllGather",
                        mybir.AluOpType.bypass,
                        replica_groups=[[0, 1], [2, 3], [4, 5], [6, 7]],
                        ins=[cc_in[c, P * d : P * (d + 1)].opt()],
                        outs=[cc_outd[c, d].opt()],
                    )
                drain_all()

            # ---- the fused pipeline ----  (xc0 DMA'd in the preamble)
            # prologue Q-projection consumes e-tiles in DMA-arrival order:
            # low halves of d0/d1 run while the high halves land.
            qt0 = qp.tile([P, ND, SC], BF16, tag="qt", name="qt0")
            qt_tiles[0] = qt0
            _accs = {}

            def _qlo(d):
                acc = psum_wk.tile([P, SC], F32, tag="wk", name="pacc")
                _accs[d] = acc
                for e in range(4):
                    nc.tensor.matmul(
                        acc[:],
                        wq_sb[:, e, P * d : P * (d + 1)],
                        xc0[:, e, :],
                        start=(e == 0),
                        stop=False,
                    )

            def _qhi(d):
                acc = _accs.pop(d)
                for e in range(4, ET):
                    nc.tensor.matmul(
                        acc[:],
                        wq_sb[:, e, P * d : P * (d + 1)],
                        xc0[:, e, :],
                        start=False,
                        stop=(e == ET - 1),
                    )
                nc.vector.tensor_copy(qt0[:, d, :], acc[:])

            for op in (lambda: _qlo(0), lambda: _qlo(1), lambda: _qhi(0),
                       lambda: _qlo(2), lambda: _qhi(1), lambda: _qlo(3),
                       lambda: _qhi(2), lambda: _qhi(3)):
                op()
            for _ in proj_gen(0, skip_q=True):
                pass
            # chunk-3's attention has the most idle PE slots (longest j-loops,
            # no projection work left), so defer outproj(1) there.
            for c in range(NSC):
                queue = deque()
                if c + 1 < NSC:
                    dma_xc(c + 1)
                    queue.append(proj_gen(c + 1))
                if c == 1:
                    queue.append(outproj_gen(0))
                elif c == 3:
                    queue.append(outproj_gen(1))
                    queue.append(outproj_gen(2))
                attention(c, queue)
            for _ in outproj_gen(NSC - 1):
                pass

    nc.compile()
    return nc


def _build_once():
    if "nc" not in _CACHE:
        _CACHE["nc"] = _build_nc()
    return _CACHE["nc"]


def _get_runner():
    """Build (once) and return a callable in_maps -> list of out_maps."""
    if "runner" in _CACHE:
        return _CACHE["runner"]

    nc = _build_once()

    import jax
    from jax.sharding import Mesh, PartitionSpec
    from jax.experimental.shard_map import shard_map
    from concourse import bass2jax
    from concourse.bass2jax import _bass_exec_p, partition_id_tensor

    bass2jax.install_neuronx_cc_hook()

    in_names, out_names, out_avals, zero_shapes = [], [], [], []
    partition_name = nc.partition_id_tensor.name if nc.partition_id_tensor else None
    for alloc in nc.m.functions[0].allocations:
        if not isinstance(alloc, mybir.MemoryLocationSet):
            continue
        name = alloc.memorylocations[0].name
        if alloc.kind == "ExternalInput":
            if name != partition_name:
                in_names.append(name)
        elif alloc.kind == "ExternalOutput":
            out_names.append(name)
            shape = tuple(alloc.tensor_shape)
            dtype = mybir.dt.np(alloc.dtype)
            out_avals.append(jax.core.ShapedArray(shape, dtype))
            zero_shapes.append((shape, dtype))
    n_params = len(in_names)
    all_in_names = list(in_names) + list(out_names)
    if partition_name is not None:
        all_in_names.append(partition_name)

    def _body(*args):
        operands = list(args)
        if partition_name is not None:
            operands.append(partition_id_tensor())
        outs = _bass_exec_p.bind(
            *operands,
            out_avals=tuple(out_avals),
            in_names=tuple(all_in_names),
            out_names=tuple(out_names),
            lowering_input_output_aliases=(),
            sim_require_finite=True,
            sim_require_nnan=True,
            nc=nc,
        )
        return tuple(outs)

    devices = jax.devices()[:NCORES]
    mesh = Mesh(np.asarray(devices), ("core",))
    n_outs = len(out_names)
    sharded = jax.jit(
        shard_map(
            _body,
            mesh=mesh,
            in_specs=(PartitionSpec("core"),) * (n_params + n_outs),
            out_specs=(PartitionSpec("core"),) * n_outs,
            check_rep=False,
        ),
        donate_argnums=tuple(range(n_params, n_params + n_outs)),
        keep_unused=True,
    )

    def runner(in_maps):
        per_core = [[np.asarray(m[name]) for name in in_names] for m in in_maps]
        concat_in = [
            np.concatenate([per_core[c][i] for c in range(NCORES)], axis=0)
            for i in range(n_params)
        ]
        concat_zeros = [
            np.zeros((NCORES * s[0], *s[1:]), d) for (s, d) in zero_shapes
        ]
        out_arrs = sharded(*concat_in, *concat_zeros)
        return [
            {
                name: np.asarray(out_arrs[i]).reshape(NCORES, *out_avals[i].shape)[c]
                for i, name in enumerate(out_names)
            }
            for c in range(NCORES)
        ]

    _CACHE["runner"] = runner
    _CACHE["sharded"] = sharded
    _CACHE["mesh"] = mesh
    _CACHE["meta"] = (in_names, out_names, zero_shapes)
    return runner


def timing_setup(in_maps):
    """Device-resident timing: returns (make_zeros, call).

    `call(make_zeros())` runs one on-device execution with inputs already
    resident (zeros are donated output buffers, created outside the timer).
    """
    _get_runner()
    import jax
    from jax.sharding import NamedSharding, PartitionSpec

    in_names, out_names, zero_shapes = _CACHE["meta"]
    sharding = NamedSharding(_CACHE["mesh"], PartitionSpec("core"))
    per_core = [[np.asarray(m[name]) for name in in_names] for m in in_maps]
    dev_in = [
        jax.device_put(
            np.concatenate([per_core[c][i] for c in range(NCORES)], axis=0), sharding
        )
        for i in range(len(in_names))
    ]
    jax.block_until_ready(dev_in)

    def make_zeros():
        zs = [
            jax.device_put(np.zeros((NCORES * s[0], *s[1:]), d), sharding)
            for (s, d) in zero_shapes
        ]
        jax.block_until_ready(zs)
        return zs

    def call(zs):
        out = _CACHE["sharded"](*dev_in, *zs)
        jax.block_until_ready(out)
        return out

    return make_zeros, call


def make_in_maps(x, Wq, Wk, Wv, Wo, bo):
    """Host-side sharding: slice/transpose full inputs into per-core maps."""
    bf16 = ml_dtypes.bfloat16
    x = np.asarray(x, dtype=np.float32)
    Wq = np.asarray(Wq, dtype=np.float32)
    Wk = np.asarray(Wk, dtype=np.float32)
    Wv = np.asarray(Wv, dtype=np.float32)
    Wo = np.asarray(Wo, dtype=np.float32)
    bo = np.asarray(bo, dtype=np.float32)

    # masks[r, i, c] = 1 iff query col c >= key row r + 128*i  (diag tile i)
    r = np.arange(P)[:, None, None]
    i = np.arange(4)[None, :, None]
    cc = np.arange(SC)[None, None, :]
    masks = (cc >= r + P * i).astype(bf16)
    # bc selector: row h is 1 on cols [64h, 64h+64)
    sel = np.zeros((2, P), bf16)
    sel[0, 0:D] = 1
    sel[1, D : 2 * D] = 1

    WoT = np.ascontiguousarray(Wo.T)
    in_maps = []
    for c in range(NCORES):
        b, g = c // 2, c % 2
        xT = np.ascontiguousarray(x[b].T).astype(bf16)
        wq = np.ascontiguousarray(
            Wq[HL * g : HL * (g + 1)].transpose(1, 0, 2).reshape(E, DG)
        ).astype(bf16)
        wk = np.ascontiguousarray(
            Wk[HL * g : HL * (g + 1)].transpose(1, 0, 2).reshape(E, DG)
        ).astype(bf16)
        wv = np.ascontiguousarray(
            Wv[HL * g : HL * (g + 1)].transpose(1, 0, 2).reshape(E, DG)
        ).astype(bf16)
        woT = WoT[:, EH * g : EH * (g + 1)].astype(bf16)
        bo_c = np.ascontiguousarray(
            bo[EH * g : EH * (g + 1)].reshape(EH // P, P).T
        )
        in_maps.append(
            {
                "xT": xT,
                "wq": wq,
                "wk": wk,
                "wv": wv,
                "woT": woT,
                "bo": bo_c,
                "masks": masks,
                "sel": sel,
            }
        )
    return in_maps


def assemble_output(results):
    """Gather per-core outT [EH, S] slices into the full [B, S, E] output."""
    out = np.empty((B, S, E), dtype=np.float32)
    for c in range(NCORES):
        b, g = c // 2, c % 2
        out[b, :, EH * g : EH * (g + 1)] = results[c]["outT"].T
    return out


def kernel(x, Wq, Wk, Wv, Wo, bo):
    runner = _get_runner()
    in_maps = make_in_maps(x, Wq, Wk, Wv, Wo, bo)
    results = runner(in_maps)
    return assemble_output(results)
